# revision 5
# baseline (speedup 1.0000x reference)
"""BDH (dense_transformer) Trainium2 kernel, 8-core tensor-parallel.

Sharding: core c -> head h=c//2, parity p=c%2. Within a head, the two cores
split the T=1024 query dim into 16 blocks of 64 (block B=2j+p, j=0..7) for
causal load balance. The N=4096 latent dim is host-permuted to [evens, odds]
so rope is pure elementwise between the E half and the O half (no pair
shuffles). Each core computes its head's full x_sparse/QR (needed for keys),
scores only for its own query columns (packed, causal-suffix structure),
yKV/y_sparse/xy/decoder for its query columns, then one 1MB AllReduce
combines the per-head partial yMLP into the replicated residual stream.

kernel(**inputs) takes full unsharded inputs, returns full (B,T,vocab) logits.
"""
import math
import sys

sys.path.insert(0, "/opt/trn_rl_repo")

import numpy as np
import ml_dtypes

import concourse.bass as bass
import concourse.mybir as mybir
import concourse.tile as tile
from concourse import bacc
from concourse.masks import make_identity
from concourse.tile import add_dep_helper
from concourse.bass_utils import run_bass_kernel_spmd

FP32 = mybir.dt.float32
BF16 = mybir.dt.bfloat16
AF = mybir.ActivationFunctionType
ALU = mybir.AluOpType

N_CORES = 8
T = 1024
D = 256
NH = 4
N = 4096
HALF = N // 2          # 2048 pairs
VOCAB = 256
EPS = 1e-5
NT = N // 128          # 32 n-tiles
NPAIR = HALF // 128    # 16 E/O tile pairs
TT8 = T // 128         # 8 t-tiles
QCOLS = 512            # per-core packed query columns
JUNK_MM = 128          # keep-warm matmuls per layer spanning the AR gap


def build(n_layer: int):
    nc = bacc.Bacc("TRN2", target_bir_lowering=False, debug=False,
                   num_devices=N_CORES)

    enc_in = nc.dram_tensor("enc", [D, N], BF16, kind="ExternalInput").ap()
    encv_in = nc.dram_tensor("encv", [D, N], BF16, kind="ExternalInput").ap()
    dec_in = nc.dram_tensor("dec", [N, D], BF16, kind="ExternalInput").ap()
    lm_in = nc.dram_tensor("lm", [D, VOCAB], BF16, kind="ExternalInput").ap()
    ctab_in = nc.dram_tensor("ctab", [HALF, T], BF16, kind="ExternalInput").ap()
    stab_in = nc.dram_tensor("stab", [HALF, T], BF16, kind="ExternalInput").ap()
    mask_in = nc.dram_tensor("mask", [128, 64], BF16, kind="ExternalInput").ap()
    m01_in = nc.dram_tensor("m01", [128, 2], FP32, kind="ExternalInput").ap()
    poff_in = nc.dram_tensor("poff", [1, 1], mybir.dt.uint32, kind="ExternalInput").ap()
    x0_in = nc.dram_tensor("x0", [T, D], FP32, kind="ExternalInput").ap()
    out = nc.dram_tensor("out", [T, VOCAB], FP32, kind="ExternalOutput").ap()

    RG = [list(range(N_CORES))]

    with tile.TileContext(nc) as tc:
        regs = nc.alloc_registers("qoff")
        nc.regs_load(regs, poff_in[0:1, 0:1])
        qoff = nc.snap(regs, donate=True, min_val=0, max_val=64)

        import contextlib
        ctx = contextlib.ExitStack()
        with ctx:
            singles = ctx.enter_context(tc.tile_pool(name="singles", bufs=1))
            big = ctx.enter_context(tc.tile_pool(name="big", bufs=1))
            work = ctx.enter_context(tc.tile_pool(name="work", bufs=2))
            small = ctx.enter_context(tc.tile_pool(name="small", bufs=2))
            stat = ctx.enter_context(tc.tile_pool(name="stat", bufs=4))
            ps = ctx.enter_context(tc.tile_pool(name="ps", bufs=1, space="PSUM"))
            dramp = ctx.enter_context(tc.tile_pool(name="dramp", bufs=2, space="DRAM"))
            spillp = ctx.enter_context(tc.tile_pool(name="spillp", bufs=34, space="DRAM"))

            # ---- persistent weights in SBUF ----
            enc_sb = singles.tile([128, 2, N], BF16)
            encv_sb = singles.tile([128, 2, N], BF16)
            dec_sb = singles.tile([128, NT, D], BF16)
            lm_sb = singles.tile([128, 2, VOCAB], BF16)
            mask_sb = singles.tile([128, 64], BF16)
            m01_sb = singles.tile([128, 2], FP32)
            eps_sb = singles.tile([128, 1], FP32)
            ident = singles.tile([128, 128], BF16)
            nc.sync.dma_start(out=enc_sb, in_=enc_in.rearrange("(kt p) n -> p kt n", p=128))
            nc.sync.dma_start(out=encv_sb, in_=encv_in.rearrange("(kt p) n -> p kt n", p=128))
            nc.sync.dma_start(out=dec_sb, in_=dec_in.rearrange("(nt p) d -> p nt d", p=128))
            nc.sync.dma_start(out=lm_sb, in_=lm_in.rearrange("(kt p) v -> p kt v", p=128))
            nc.sync.dma_start(out=mask_sb, in_=mask_in)
            nc.sync.dma_start(out=m01_sb, in_=m01_in)
            nc.vector.memset(eps_sb, EPS)
            make_identity(nc, ident)

            # ---- persistent activations ----
            x_sb = big.tile([128, TT8, D], FP32)     # residual, t-part layout
            xbf_sb = big.tile([128, TT8, D], BF16)
            xT_sb = big.tile([128, 2, T], BF16)      # d-part layout
            qr_sb = big.tile([128, NT, T], BF16)     # QR, n-part layout
            a_sb = big.tile([128, TT8, QCOLS], BF16)  # masked scores lhsT
            ykv_sb = big.tile([128, 4, D], BF16)
            ykvT_sb = big.tile([128, 2, QCOLS], BF16)

            def layernorm(dst, src, tmp_pool):
                """dst = LN(src) over free dim (free size = D). src/dst (128, D)."""
                stats = tmp_pool.tile([128, 6], FP32, tag="lnstats")
                mv = tmp_pool.tile([128, 2], FP32, tag="lnmv")
                nc.vector.bn_stats(out=stats, in_=src)
                nc.vector.bn_aggr(out=mv, in_=stats)
                std_t = tmp_pool.tile([128, 1], FP32, tag="lnstd")
                nc.scalar.activation(out=std_t, in_=mv[:, 1:2], func=AF.Sqrt,
                                     bias=eps_sb, scale=1.0)
                rstd = tmp_pool.tile([128, 1], FP32, tag="lnrstd")
                nc.vector.reciprocal(out=rstd, in_=std_t)
                negmr = tmp_pool.tile([128, 1], FP32, tag="lnnegmr")
                nc.vector.tensor_scalar(out=negmr, in0=mv[:, 0:1], scalar1=rstd,
                                        scalar2=-1.0, op0=ALU.mult, op1=ALU.mult)
                nc.scalar.activation(out=dst, in_=src, func=AF.Identity,
                                     bias=negmr, scale=rstd)

            def x_finalize(ti):
                """cast x_sb[:, ti] -> xbf, transpose into xT_sb."""
                nc.scalar.copy(out=xbf_sb[:, ti, :], in_=x_sb[:, ti, :])
                for dh in range(2):
                    pst = ps.tile([128, 128], BF16, tag=f"b{2 + dh}", name=f"tpose{ti}_{dh}")
                    nc.tensor.transpose(pst[:], xbf_sb[:, ti, 128 * dh:128 * dh + 128], ident[:])
                    nc.scalar.copy(out=xT_sb[:, dh, 128 * ti:128 * ti + 128], in_=pst[:])

            # warmup collective: absorbs the one-time global sync barrier
            wu_in = dramp.tile([128, 4], FP32, tag="wuin")
            wu_out = dramp.tile([128, 4], FP32, tag="wuout")
            wu_sb = small.tile([128, 4], FP32, tag="wusb")
            nc.vector.memset(wu_sb, 0.0)
            nc.sync.dma_start(out=wu_in[:], in_=wu_sb)
            nc.gpsimd.collective_compute("AllReduce", ALU.add, replica_groups=RG,
                                         ins=[wu_in.opt()], outs=[wu_out.opt()])

            # ---- prologue: x = LN(x0) ----
            for ti in range(TT8):
                raw = small.tile([128, D], FP32, tag="x0raw")
                nc.sync.dma_start(out=raw, in_=x0_in[128 * ti:128 * ti + 128, :])
                layernorm(x_sb[:, ti, :], raw, stat)
                x_finalize(ti)

            # ---- layers ----
            for ell in range(n_layer):
                # Phase A: x_sparse (per E/O pair), spill packed q-cols, rope -> QR
                spills = []
                for i in range(NPAIR):
                    b4 = 4 * (i % 2)
                    psE = [ps.tile([128, 512], FP32, tag=f"b{b4 + ch}", name=f"psE{i}_{ch}") for ch in range(2)]
                    psO = [ps.tile([128, 512], FP32, tag=f"b{b4 + 2 + ch}", name=f"psO{i}_{ch}") for ch in range(2)]
                    for ch in range(2):
                        for kt in range(2):
                            nc.tensor.matmul(psE[ch][:],
                                             enc_sb[:, kt, 128 * i:128 * i + 128],
                                             xT_sb[:, kt, 512 * ch:512 * ch + 512],
                                             start=(kt == 0), stop=(kt == 1))
                            nc.tensor.matmul(psO[ch][:],
                                             enc_sb[:, kt, HALF + 128 * i:HALF + 128 * i + 128],
                                             xT_sb[:, kt, 512 * ch:512 * ch + 512],
                                             start=(kt == 0), stop=(kt == 1))
                    xsE = work.tile([128, T], BF16, tag="xsE")
                    xsO = work.tile([128, T], BF16, tag="xsO")
                    for ch in range(2):
                        nc.scalar.activation(out=xsE[:, 512 * ch:512 * ch + 512],
                                             in_=psE[ch][:], func=AF.Relu)
                        nc.scalar.activation(out=xsO[:, 512 * ch:512 * ch + 512],
                                             in_=psO[ch][:], func=AF.Relu)
                    # spill packed q cols (pre-rope) for xy later
                    for half, xs in ((0, xsE), (1, xsO)):
                        sp = spillp.tile([128, QCOLS], BF16, tag="xsq")
                        nc.gpsimd.dma_start(
                            out=sp[:].rearrange("p (b w) -> p b w", w=64),
                            in_=xs[:].rearrange("p (b w) -> p b w", w=128)[:, :, bass.ds(qoff, 64)])
                        spills.append((16 * half + i, sp))
                    # rope: QR_E = xsE*c - xsO*s ; QR_O = xsO*c + xsE*s
                    ct = small.tile([128, T], BF16, tag="ctab")
                    st = small.tile([128, T], BF16, tag="stab")
                    nc.sync.dma_start(out=ct, in_=ctab_in[128 * i:128 * i + 128, :])
                    nc.sync.dma_start(out=st, in_=stab_in[128 * i:128 * i + 128, :])
                    tB = work.tile([128, T], BF16, tag="tB")
                    tD = work.tile([128, T], BF16, tag="tD")
                    qrE = qr_sb[:, i, :]
                    qrO = qr_sb[:, 16 + i, :]
                    nc.gpsimd.tensor_tensor(out=tB, in0=xsO, in1=st, op=ALU.mult)
                    nc.gpsimd.tensor_tensor(out=tD, in0=xsE, in1=st, op=ALU.mult)
                    nc.vector.tensor_tensor(out=qrE, in0=xsE, in1=ct, op=ALU.mult)
                    nc.vector.tensor_tensor(out=qrO, in0=xsO, in1=ct, op=ALU.mult)
                    nc.vector.tensor_tensor(out=qrE, in0=qrE, in1=tB, op=ALU.subtract)
                    nc.vector.tensor_tensor(out=qrO, in0=qrO, in1=tD, op=ALU.add)
                spill_map = dict(spills)

                # Phase B: scores. bank t: S[s_tile=t rows, packed q cols 64t:512]
                nc.gpsimd.memset(a_sb[:], 0.0)
                for t in range(TT8):
                    Ft = QCOLS - 64 * t
                    pss = ps.tile([128, Ft], FP32, tag=f"b{t}", name=f"sps{ell}_{t}")
                    for n in range(NT):
                        rhs = qr_sb[:, n, :].rearrange("p (b w) -> p b w", w=128)[:, t:TT8, bass.ds(qoff, 64)]
                        nc.tensor.matmul(pss[:].rearrange("p (b w) -> p b w", w=64),
                                         qr_sb[:, n, 128 * t:128 * t + 128],
                                         rhs, start=(n == 0), stop=(n == NT - 1))
                    # eviction: diagonal block masked, rest plain, below-diag stays 0
                    nc.vector.tensor_tensor(out=a_sb[:, t, 64 * t:64 * t + 64],
                                            in0=pss[:, 0:64], in1=mask_sb, op=ALU.mult)
                    if t < TT8 - 1:
                        nc.scalar.copy(out=a_sb[:, t, 64 * t + 64:QCOLS],
                                       in_=pss[:, 64:Ft])

                # Phase C: yKV (packed q rows) + LN + transpose
                for k in range(4):
                    psy = ps.tile([128, D], FP32, tag=f"b{k % 2}", name=f"ykvps{ell}_{k}")
                    tmax = 2 * k + 1
                    for t in range(tmax + 1):
                        nc.tensor.matmul(psy[:],
                                         a_sb[:, t, 128 * k:128 * k + 128],
                                         xbf_sb[:, t, :],
                                         start=(t == 0), stop=(t == tmax))
                    layernorm(ykv_sb[:, k, :], psy[:], stat)
                    for dh in range(2):
                        pst = ps.tile([128, 128], BF16, tag=f"b{2 + dh}", name=f"ykvT{ell}_{k}_{dh}")
                        nc.tensor.transpose(pst[:], ykv_sb[:, k, 128 * dh:128 * dh + 128], ident[:])
                        nc.scalar.copy(out=ykvT_sb[:, dh, 128 * k:128 * k + 128], in_=pst[:])

                # Phase D: y_sparse, xy, decoder partials
                psmlp = [ps.tile([128, D], FP32, tag=f"b{4 + k}", name=f"psmlp{ell}_{k}") for k in range(4)]
                for j in range(NT):
                    psy = ps.tile([128, QCOLS], FP32, tag=f"b{j % 4}", name=f"ysps{ell}_{j}")
                    for kt in range(2):
                        nc.tensor.matmul(psy[:],
                                         encv_sb[:, kt, 128 * j:128 * j + 128],
                                         ykvT_sb[:, kt, :],
                                         start=(kt == 0), stop=(kt == 1))
                    ys = small.tile([128, QCOLS], BF16, tag="ys")
                    nc.scalar.activation(out=ys, in_=psy[:], func=AF.Relu)
                    xsq = small.tile([128, QCOLS], BF16, tag="xsqb")
                    nc.sync.dma_start(out=xsq, in_=spill_map[j][:])
                    xy = small.tile([128, QCOLS], BF16, tag="xy")
                    nc.vector.tensor_tensor(out=xy, in0=ys, in1=xsq, op=ALU.mult)
                    for k in range(4):
                        nc.tensor.matmul(psmlp[k][:],
                                         xy[:, 128 * k:128 * k + 128],
                                         dec_sb[:, j, :],
                                         start=(j == 0), stop=(j == NT - 1))

                # keep-warm junk matmuls: span the AR + x-update gap so the
                # PE activity monitor doesn't re-throttle the clock
                junk = ps.tile([128, 512], FP32, tag="b0", name=f"junk{ell}")
                for jm in range(JUNK_MM):
                    nc.tensor.matmul(junk[:], enc_sb[:, 0, 0:128],
                                     enc_sb[:, 1, 0:512],
                                     start=(jm == 0), stop=(jm == JUNK_MM - 1))

                # Phase E: AR of yMLP, x update
                ar_in = dramp.tile([T, D], FP32, tag="arin")
                ar_out = dramp.tile([T, D], FP32, tag="arout")
                wds = []
                for P in (0, 1):
                    for k in range(4):
                        ym = stat.tile([128, D], FP32, tag="ymsk")
                        nc.vector.tensor_scalar_mul(out=ym, in0=psmlp[k][:],
                                                    scalar1=m01_sb[:, P:P + 1])
                        for aa in range(2):
                            out_ap = bass.AP(
                                tensor=ar_in.tensor,
                                offset=ar_in.offset + (256 * k + 64 * P + 128 * aa) * D,
                                ap=[[D, 64], [1, D]],
                            )
                            wds.append(nc.sync.dma_start(
                                out=out_ap, in_=ym[64 * aa:64 * aa + 64, :]))
                cc = nc.gpsimd.collective_compute(
                    "AllReduce", ALU.add, replica_groups=RG,
                    ins=[ar_in.opt()], outs=[ar_out.opt()])
                for w in wds:
                    add_dep_helper(cc.ins, w.ins, sync=True, reason="w->ar")
                for ti in range(TT8):
                    rd = stat.tile([128, D], FP32, tag="ymrd")
                    rdma = nc.sync.dma_start(out=rd, in_=ar_out[128 * ti:128 * ti + 128, :])
                    add_dep_helper(rdma.ins, cc.ins, sync=True, reason="ar->r")
                    lnym = stat.tile([128, D], FP32, tag="lnym")
                    layernorm(lnym, rd, stat)
                    xn = stat.tile([128, D], FP32, tag="xn")
                    nc.vector.tensor_add(out=xn, in0=x_sb[:, ti, :], in1=lnym)
                    layernorm(x_sb[:, ti, :], xn, stat)
                    x_finalize(ti)

            # ---- logits ----
            for ti in range(TT8):
                psl = ps.tile([128, VOCAB], FP32, tag=f"b{ti % 2}", name=f"lgps{ti}")
                for kt in range(2):
                    nc.tensor.matmul(psl[:],
                                     xT_sb[:, kt, 128 * ti:128 * ti + 128],
                                     lm_sb[:, kt, :],
                                     start=(kt == 0), stop=(kt == 1))
                lg = small.tile([128, VOCAB], FP32, tag="lg")
                nc.scalar.copy(out=lg, in_=psl[:])
                nc.sync.dma_start(out=out[128 * ti:128 * ti + 128, :], in_=lg)

    nc.compile()
    return nc


def _host_prep(idx, embed_w, encoder, encoder_v, decoder, lm_head):
    """Build the 8 per-core input maps."""
    idx = np.asarray(idx)
    B, Tt = idx.shape
    assert B == 1 and Tt == T
    perm = np.concatenate([np.arange(0, N, 2), np.arange(1, N, 2)])

    def bf(x):
        return np.ascontiguousarray(x).astype(ml_dtypes.bfloat16)

    # rope tables for pair k (even original index 2k), fp32 math like reference
    theta = 2.0 ** 16
    q = np.floor(np.arange(N, dtype=np.float32) / 2.0) * 2.0
    freqs = (1.0 / (theta ** (q / np.float32(N))) / np.float32(2.0 * math.pi)).astype(np.float32)
    phases = np.arange(T, dtype=np.float32)[:, None] * freqs[None, 0::2]  # (T, HALF)
    ph = np.float32(2.0 * math.pi) * (phases % np.float32(1.0))
    ctab = bf(np.cos(ph).T)   # (HALF, T)
    stab = bf(np.sin(ph).T)

    x0 = np.ascontiguousarray(embed_w[idx[0]]).astype(np.float32)  # (T, D)
    lm_bf = bf(lm_head)

    r = np.arange(128)[:, None]
    c64 = np.arange(64)[None, :]
    in_maps = []
    for c in range(N_CORES):
        h, p = c // 2, c % 2
        m01 = np.zeros((128, 2), np.float32)
        m01[:, 0] = 1.0 - p
        m01[:, 1] = p
        in_maps.append({
            "enc": bf(encoder[h][:, perm]),
            "encv": bf(encoder_v[h][:, perm]),
            "dec": bf(decoder.reshape(NH, N, D)[h][perm, :]),
            "lm": lm_bf,
            "ctab": ctab,
            "stab": stab,
            "mask": bf((r < c64 + 64 * p).astype(np.float32)),
            "m01": m01,
            "poff": np.array([[64 * p]], dtype=np.uint32),
            "x0": x0,
        })
    return in_maps


_NC_CACHE = {}


def kernel(idx, n_layer, embed_w, encoder, encoder_v, decoder, lm_head,
           _trace=False, _trace_kwargs=None):
    n_layer = int(np.asarray(n_layer))
    idx = np.asarray(idx)
    B = idx.shape[0]
    if n_layer not in _NC_CACHE:
        _NC_CACHE[n_layer] = build(n_layer)
    nc = _NC_CACHE[n_layer]
    in_maps = _host_prep(idx, np.asarray(embed_w, np.float32),
                         np.asarray(encoder, np.float32),
                         np.asarray(encoder_v, np.float32),
                         np.asarray(decoder, np.float32),
                         np.asarray(lm_head, np.float32))
    kw = {}
    if _trace:
        kw = dict(trace=True, **(_trace_kwargs or {}))
    res = run_bass_kernel_spmd(nc, in_maps, core_ids=list(range(N_CORES)), **kw)
    logits = res.results[0]["out"].astype(np.float32).reshape(B, T, VOCAB)
    kernel._last_results = res
    return logits


# revision 6
# speedup vs baseline: 1.2572x; 1.2572x over previous
"""BDH (dense_transformer) Trainium2 kernel, 8-core tensor-parallel.

Sharding: core c -> head h=c//2, parity p=c%2. Within a head, the two cores
split the T=1024 query dim into 16 blocks of 64 (block B=2j+p, j=0..7) for
causal load balance. The N=4096 latent dim is host-permuted to [evens, odds]
so rope is pure elementwise between the E half and the O half (no pair
shuffles). Each core computes its head's full x_sparse/QR (needed for keys),
scores only for its own query columns (packed, causal-suffix structure),
yKV/y_sparse/xy/decoder for its query columns, then one 1MB AllReduce
combines the per-head partial yMLP into the replicated residual stream.

kernel(**inputs) takes full unsharded inputs, returns full (B,T,vocab) logits.
"""
import math
import sys

sys.path.insert(0, "/opt/trn_rl_repo")

import numpy as np
import ml_dtypes

import concourse.bass as bass
import concourse.mybir as mybir
import concourse.tile as tile
from concourse import bacc
from concourse.masks import make_identity
from concourse.tile import add_dep_helper
from concourse.bass_utils import run_bass_kernel_spmd

FP32 = mybir.dt.float32
BF16 = mybir.dt.bfloat16
AF = mybir.ActivationFunctionType
ALU = mybir.AluOpType

N_CORES = 8
T = 1024
D = 256
NH = 4
N = 4096
HALF = N // 2          # 2048 pairs
VOCAB = 256
EPS = 1e-5
NT = N // 128          # 32 n-tiles
NPAIR = HALF // 128    # 16 E/O tile pairs
TT8 = T // 128         # 8 t-tiles
QCOLS = 512            # per-core packed query columns
JUNK_MM = 64          # keep-warm matmuls per layer spanning the AR gap


def build(n_layer: int):
    nc = bacc.Bacc("TRN2", target_bir_lowering=False, debug=False,
                   num_devices=N_CORES)

    enc_in = nc.dram_tensor("enc", [D, N], BF16, kind="ExternalInput").ap()
    encv_in = nc.dram_tensor("encv", [D, N], BF16, kind="ExternalInput").ap()
    dec_in = nc.dram_tensor("dec", [N, D], BF16, kind="ExternalInput").ap()
    lm_in = nc.dram_tensor("lm", [D, VOCAB], BF16, kind="ExternalInput").ap()
    ctab_in = nc.dram_tensor("ctab", [HALF, T], BF16, kind="ExternalInput").ap()
    stab_in = nc.dram_tensor("stab", [HALF, T], BF16, kind="ExternalInput").ap()
    mask_in = nc.dram_tensor("mask", [128, 64], BF16, kind="ExternalInput").ap()
    m01_in = nc.dram_tensor("m01", [128, 2], FP32, kind="ExternalInput").ap()
    poff_in = nc.dram_tensor("poff", [1, 1], mybir.dt.uint32, kind="ExternalInput").ap()
    x0_in = nc.dram_tensor("x0", [T, D], FP32, kind="ExternalInput").ap()
    out = nc.dram_tensor("out", [T, VOCAB], FP32, kind="ExternalOutput").ap()

    RG = [list(range(N_CORES))]

    with tile.TileContext(nc) as tc:
        regs = nc.alloc_registers("qoff")
        nc.regs_load(regs, poff_in[0:1, 0:1])
        qoff = nc.snap(regs, donate=True, min_val=0, max_val=64)

        import contextlib
        ctx = contextlib.ExitStack()
        with ctx:
            singles = ctx.enter_context(tc.tile_pool(name="singles", bufs=1))
            big = ctx.enter_context(tc.tile_pool(name="big", bufs=1))
            work = ctx.enter_context(tc.tile_pool(name="work", bufs=2))
            small = ctx.enter_context(tc.tile_pool(name="small", bufs=2))
            stat = ctx.enter_context(tc.tile_pool(name="stat", bufs=4))
            ps = ctx.enter_context(tc.tile_pool(name="ps", bufs=1, space="PSUM"))
            dramp = ctx.enter_context(tc.tile_pool(name="dramp", bufs=2, space="DRAM"))
            spillp = ctx.enter_context(tc.tile_pool(name="spillp", bufs=34, space="DRAM"))

            # ---- persistent weights in SBUF ----
            enc_sb = singles.tile([128, 2, N], BF16)
            encv_sb = singles.tile([128, 2, N], BF16)
            dec_sb = singles.tile([128, NT, D], BF16)
            lm_sb = singles.tile([128, 2, VOCAB], BF16)
            mask_sb = singles.tile([128, 64], BF16)
            m01_sb = singles.tile([128, 2], FP32)
            eps_sb = singles.tile([128, 1], FP32)
            ident = singles.tile([128, 128], BF16)
            nc.sync.dma_start(out=enc_sb, in_=enc_in.rearrange("(kt p) n -> p kt n", p=128))
            nc.sync.dma_start(out=encv_sb, in_=encv_in.rearrange("(kt p) n -> p kt n", p=128))
            nc.sync.dma_start(out=dec_sb, in_=dec_in.rearrange("(nt p) d -> p nt d", p=128))
            nc.sync.dma_start(out=lm_sb, in_=lm_in.rearrange("(kt p) v -> p kt v", p=128))
            nc.sync.dma_start(out=mask_sb, in_=mask_in)
            nc.sync.dma_start(out=m01_sb, in_=m01_in)
            nc.vector.memset(eps_sb, EPS)
            make_identity(nc, ident)

            # ---- persistent activations ----
            x_sb = big.tile([128, TT8, D], FP32)     # residual, t-part layout
            xbf_sb = big.tile([128, TT8, D], BF16)
            xT_sb = big.tile([128, 2, T], BF16)      # d-part layout
            qr_sb = big.tile([128, NT, T], BF16)     # QR, n-part layout
            a_sb = big.tile([128, TT8, QCOLS], BF16)  # masked scores lhsT
            ykv_sb = big.tile([128, 4, D], BF16)
            ykvT_sb = big.tile([128, 2, QCOLS], BF16)

            def layernorm(dst, src, tmp_pool):
                """dst = LN(src) over free dim (free size = D). src/dst (128, D)."""
                stats = tmp_pool.tile([128, 6], FP32, tag="lnstats")
                mv = tmp_pool.tile([128, 2], FP32, tag="lnmv")
                nc.vector.bn_stats(out=stats, in_=src)
                nc.vector.bn_aggr(out=mv, in_=stats)
                std_t = tmp_pool.tile([128, 1], FP32, tag="lnstd")
                nc.scalar.activation(out=std_t, in_=mv[:, 1:2], func=AF.Sqrt,
                                     bias=eps_sb, scale=1.0)
                rstd = tmp_pool.tile([128, 1], FP32, tag="lnrstd")
                nc.vector.reciprocal(out=rstd, in_=std_t)
                negmr = tmp_pool.tile([128, 1], FP32, tag="lnnegmr")
                nc.vector.tensor_scalar(out=negmr, in0=mv[:, 0:1], scalar1=rstd,
                                        scalar2=-1.0, op0=ALU.mult, op1=ALU.mult)
                nc.scalar.activation(out=dst, in_=src, func=AF.Identity,
                                     bias=negmr, scale=rstd)

            def x_finalize(ti):
                """cast x_sb[:, ti] -> xbf, transpose into xT_sb."""
                nc.scalar.copy(out=xbf_sb[:, ti, :], in_=x_sb[:, ti, :])
                for dh in range(2):
                    pst = ps.tile([128, 128], BF16, tag=f"b{2 + dh}", name=f"tpose{ti}_{dh}")
                    nc.tensor.transpose(pst[:], xbf_sb[:, ti, 128 * dh:128 * dh + 128], ident[:])
                    nc.scalar.copy(out=xT_sb[:, dh, 128 * ti:128 * ti + 128], in_=pst[:])

            # warmup collective: absorbs the one-time global sync barrier
            wu_in = dramp.tile([128, 4], FP32, tag="wuin")
            wu_out = dramp.tile([128, 4], FP32, tag="wuout")
            wu_sb = small.tile([128, 4], FP32, tag="wusb")
            nc.vector.memset(wu_sb, 0.0)
            nc.sync.dma_start(out=wu_in[:], in_=wu_sb)
            nc.gpsimd.collective_compute("AllReduce", ALU.add, replica_groups=RG,
                                         ins=[wu_in.opt()], outs=[wu_out.opt()])

            # ---- prologue: x = LN(x0) ----
            for ti in range(TT8):
                raw = small.tile([128, D], FP32, tag="x0raw")
                nc.sync.dma_start(out=raw, in_=x0_in[128 * ti:128 * ti + 128, :])
                layernorm(x_sb[:, ti, :], raw, stat)
                x_finalize(ti)

            # ---- layers ----
            for ell in range(n_layer):
                # Phase A: x_sparse (per E/O pair), spill packed q-cols, rope -> QR
                spills = []
                for i in range(NPAIR):
                    b4 = 4 * (i % 2)
                    psE = [ps.tile([128, 512], FP32, tag=f"b{b4 + ch}", name=f"psE{i}_{ch}") for ch in range(2)]
                    psO = [ps.tile([128, 512], FP32, tag=f"b{b4 + 2 + ch}", name=f"psO{i}_{ch}") for ch in range(2)]
                    for ch in range(2):
                        for kt in range(2):
                            nc.tensor.matmul(psE[ch][:],
                                             enc_sb[:, kt, 128 * i:128 * i + 128],
                                             xT_sb[:, kt, 512 * ch:512 * ch + 512],
                                             start=(kt == 0), stop=(kt == 1))
                            nc.tensor.matmul(psO[ch][:],
                                             enc_sb[:, kt, HALF + 128 * i:HALF + 128 * i + 128],
                                             xT_sb[:, kt, 512 * ch:512 * ch + 512],
                                             start=(kt == 0), stop=(kt == 1))
                    xsE = work.tile([128, T], BF16, tag="xsE")
                    xsO = work.tile([128, T], BF16, tag="xsO")
                    for ch in range(2):
                        nc.scalar.activation(out=xsE[:, 512 * ch:512 * ch + 512],
                                             in_=psE[ch][:], func=AF.Relu)
                        nc.scalar.activation(out=xsO[:, 512 * ch:512 * ch + 512],
                                             in_=psO[ch][:], func=AF.Relu)
                    # spill packed q cols (pre-rope) for xy later
                    for half, xs in ((0, xsE), (1, xsO)):
                        sp = spillp.tile([128, QCOLS], BF16, tag="xsq")
                        nc.gpsimd.dma_start(
                            out=sp[:].rearrange("p (b w) -> p b w", w=64),
                            in_=xs[:].rearrange("p (b w) -> p b w", w=128)[:, :, bass.ds(qoff, 64)])
                        spills.append((16 * half + i, sp))
                    # rope: QR_E = xsE*c - xsO*s ; QR_O = xsO*c + xsE*s
                    ct = small.tile([128, T], BF16, tag="ctab")
                    st = small.tile([128, T], BF16, tag="stab")
                    nc.sync.dma_start(out=ct, in_=ctab_in[128 * i:128 * i + 128, :])
                    nc.sync.dma_start(out=st, in_=stab_in[128 * i:128 * i + 128, :])
                    tB = work.tile([128, T], BF16, tag="tB")
                    tD = work.tile([128, T], BF16, tag="tD")
                    qrE = qr_sb[:, i, :]
                    qrO = qr_sb[:, 16 + i, :]
                    nc.vector.tensor_tensor(out=tB, in0=xsO, in1=st, op=ALU.mult)
                    nc.vector.tensor_tensor(out=tD, in0=xsE, in1=st, op=ALU.mult)
                    nc.vector.tensor_tensor(out=qrE, in0=xsE, in1=ct, op=ALU.mult)
                    nc.vector.tensor_tensor(out=qrO, in0=xsO, in1=ct, op=ALU.mult)
                    nc.vector.tensor_tensor(out=qrE, in0=qrE, in1=tB, op=ALU.subtract)
                    nc.vector.tensor_tensor(out=qrO, in0=qrO, in1=tD, op=ALU.add)
                spill_map = dict(spills)

                # Phase B: scores. bank t: S[s_tile=t rows, packed q cols 64t:512]
                nc.gpsimd.memset(a_sb[:], 0.0)
                for t in range(TT8):
                    Ft = QCOLS - 64 * t
                    pss = ps.tile([128, Ft], FP32, tag=f"b{t}", name=f"sps{ell}_{t}")
                    for n in range(NT):
                        rhs = qr_sb[:, n, :].rearrange("p (b w) -> p b w", w=128)[:, t:TT8, bass.ds(qoff, 64)]
                        nc.tensor.matmul(pss[:].rearrange("p (b w) -> p b w", w=64),
                                         qr_sb[:, n, 128 * t:128 * t + 128],
                                         rhs, start=(n == 0), stop=(n == NT - 1))
                    # eviction: diagonal block masked, rest plain, below-diag stays 0
                    nc.vector.tensor_tensor(out=a_sb[:, t, 64 * t:64 * t + 64],
                                            in0=pss[:, 0:64], in1=mask_sb, op=ALU.mult)
                    if t < TT8 - 1:
                        nc.scalar.copy(out=a_sb[:, t, 64 * t + 64:QCOLS],
                                       in_=pss[:, 64:Ft])

                # Phase C: yKV (packed q rows) + LN + transpose
                for k in range(4):
                    psy = ps.tile([128, D], FP32, tag=f"b{k % 2}", name=f"ykvps{ell}_{k}")
                    tmax = 2 * k + 1
                    for t in range(tmax + 1):
                        nc.tensor.matmul(psy[:],
                                         a_sb[:, t, 128 * k:128 * k + 128],
                                         xbf_sb[:, t, :],
                                         start=(t == 0), stop=(t == tmax))
                    layernorm(ykv_sb[:, k, :], psy[:], stat)
                    for dh in range(2):
                        pst = ps.tile([128, 128], BF16, tag=f"b{2 + dh}", name=f"ykvT{ell}_{k}_{dh}")
                        nc.tensor.transpose(pst[:], ykv_sb[:, k, 128 * dh:128 * dh + 128], ident[:])
                        nc.scalar.copy(out=ykvT_sb[:, dh, 128 * k:128 * k + 128], in_=pst[:])

                # Phase D: y_sparse, xy, decoder partials
                psmlp = [ps.tile([128, D], FP32, tag=f"b{4 + k}", name=f"psmlp{ell}_{k}") for k in range(4)]
                for j in range(NT):
                    psy = ps.tile([128, QCOLS], FP32, tag=f"b{j % 4}", name=f"ysps{ell}_{j}")
                    for kt in range(2):
                        nc.tensor.matmul(psy[:],
                                         encv_sb[:, kt, 128 * j:128 * j + 128],
                                         ykvT_sb[:, kt, :],
                                         start=(kt == 0), stop=(kt == 1))
                    ys = small.tile([128, QCOLS], BF16, tag="ys")
                    nc.scalar.activation(out=ys, in_=psy[:], func=AF.Relu)
                    xsq = small.tile([128, QCOLS], BF16, tag="xsqb")
                    nc.sync.dma_start(out=xsq, in_=spill_map[j][:])
                    xy = small.tile([128, QCOLS], BF16, tag="xy")
                    nc.vector.tensor_tensor(out=xy, in0=ys, in1=xsq, op=ALU.mult)
                    for k in range(4):
                        nc.tensor.matmul(psmlp[k][:],
                                         xy[:, 128 * k:128 * k + 128],
                                         dec_sb[:, j, :],
                                         start=(j == 0), stop=(j == NT - 1))

                # keep-warm junk matmuls: span the AR + x-update gap so the
                # PE activity monitor doesn't re-throttle the clock
                junk = ps.tile([128, 512], FP32, tag="b0", name=f"junk{ell}")
                for jm in range(JUNK_MM):
                    nc.tensor.matmul(junk[:], enc_sb[:, 0, 0:128],
                                     enc_sb[:, 1, 0:512],
                                     start=(jm == 0), stop=(jm == JUNK_MM - 1))

                # Phase E: AR of yMLP, x update
                ar_in = dramp.tile([T, D], FP32, tag="arin")
                ar_out = dramp.tile([T, D], FP32, tag="arout")
                wds = []
                for P in (0, 1):
                    for k in range(4):
                        ym = stat.tile([128, D], FP32, tag="ymsk")
                        nc.vector.tensor_scalar_mul(out=ym, in0=psmlp[k][:],
                                                    scalar1=m01_sb[:, P:P + 1])
                        for aa in range(2):
                            out_ap = bass.AP(
                                tensor=ar_in.tensor,
                                offset=ar_in.offset + (256 * k + 64 * P + 128 * aa) * D,
                                ap=[[D, 64], [1, D]],
                            )
                            wds.append(nc.sync.dma_start(
                                out=out_ap, in_=ym[64 * aa:64 * aa + 64, :]))
                cc = nc.gpsimd.collective_compute(
                    "AllReduce", ALU.add, replica_groups=RG,
                    ins=[ar_in.opt()], outs=[ar_out.opt()])
                for w in wds:
                    add_dep_helper(cc.ins, w.ins, sync=True, reason="w->ar")
                for ti in range(TT8):
                    rd = stat.tile([128, D], FP32, tag="ymrd")
                    rdma = nc.sync.dma_start(out=rd, in_=ar_out[128 * ti:128 * ti + 128, :])
                    add_dep_helper(rdma.ins, cc.ins, sync=True, reason="ar->r")
                    lnym = stat.tile([128, D], FP32, tag="lnym")
                    layernorm(lnym, rd, stat)
                    xn = stat.tile([128, D], FP32, tag="xn")
                    nc.vector.tensor_add(out=xn, in0=x_sb[:, ti, :], in1=lnym)
                    layernorm(x_sb[:, ti, :], xn, stat)
                    x_finalize(ti)

            # ---- logits ----
            for ti in range(TT8):
                psl = ps.tile([128, VOCAB], FP32, tag=f"b{ti % 2}", name=f"lgps{ti}")
                for kt in range(2):
                    nc.tensor.matmul(psl[:],
                                     xT_sb[:, kt, 128 * ti:128 * ti + 128],
                                     lm_sb[:, kt, :],
                                     start=(kt == 0), stop=(kt == 1))
                lg = small.tile([128, VOCAB], FP32, tag="lg")
                nc.scalar.copy(out=lg, in_=psl[:])
                nc.sync.dma_start(out=out[128 * ti:128 * ti + 128, :], in_=lg)

    nc.compile()
    return nc


def _host_prep(idx, embed_w, encoder, encoder_v, decoder, lm_head):
    """Build the 8 per-core input maps."""
    idx = np.asarray(idx)
    B, Tt = idx.shape
    assert B == 1 and Tt == T
    perm = np.concatenate([np.arange(0, N, 2), np.arange(1, N, 2)])

    def bf(x):
        return np.ascontiguousarray(x).astype(ml_dtypes.bfloat16)

    # rope tables for pair k (even original index 2k), fp32 math like reference
    theta = 2.0 ** 16
    q = np.floor(np.arange(N, dtype=np.float32) / 2.0) * 2.0
    freqs = (1.0 / (theta ** (q / np.float32(N))) / np.float32(2.0 * math.pi)).astype(np.float32)
    phases = np.arange(T, dtype=np.float32)[:, None] * freqs[None, 0::2]  # (T, HALF)
    ph = np.float32(2.0 * math.pi) * (phases % np.float32(1.0))
    ctab = bf(np.cos(ph).T)   # (HALF, T)
    stab = bf(np.sin(ph).T)

    x0 = np.ascontiguousarray(embed_w[idx[0]]).astype(np.float32)  # (T, D)
    lm_bf = bf(lm_head)

    r = np.arange(128)[:, None]
    c64 = np.arange(64)[None, :]
    in_maps = []
    for c in range(N_CORES):
        h, p = c // 2, c % 2
        m01 = np.zeros((128, 2), np.float32)
        m01[:, 0] = 1.0 - p
        m01[:, 1] = p
        in_maps.append({
            "enc": bf(encoder[h][:, perm]),
            "encv": bf(encoder_v[h][:, perm]),
            "dec": bf(decoder.reshape(NH, N, D)[h][perm, :]),
            "lm": lm_bf,
            "ctab": ctab,
            "stab": stab,
            "mask": bf((r < c64 + 64 * p).astype(np.float32)),
            "m01": m01,
            "poff": np.array([[64 * p]], dtype=np.uint32),
            "x0": x0,
        })
    return in_maps


_NC_CACHE = {}


def kernel(idx, n_layer, embed_w, encoder, encoder_v, decoder, lm_head,
           _trace=False, _trace_kwargs=None):
    n_layer = int(np.asarray(n_layer))
    idx = np.asarray(idx)
    B = idx.shape[0]
    if n_layer not in _NC_CACHE:
        _NC_CACHE[n_layer] = build(n_layer)
    nc = _NC_CACHE[n_layer]
    in_maps = _host_prep(idx, np.asarray(embed_w, np.float32),
                         np.asarray(encoder, np.float32),
                         np.asarray(encoder_v, np.float32),
                         np.asarray(decoder, np.float32),
                         np.asarray(lm_head, np.float32))
    kw = {}
    if _trace:
        kw = dict(trace=True, **(_trace_kwargs or {}))
    res = run_bass_kernel_spmd(nc, in_maps, core_ids=list(range(N_CORES)), **kw)
    logits = res.results[0]["out"].astype(np.float32).reshape(B, T, VOCAB)
    kernel._last_results = res
    return logits
